# revision 7
# baseline (speedup 1.0000x reference)
"""Trainium2 Bass kernel for nn_Attention_65403761984268.

Causal attention with additive score bias, returning (out, sim):
    qkv = x @ Wqkv ; q,k,v = split(qkv)
    sim = q @ k^T * d^-0.5 + prev, masked (strict upper) with -FLT_MAX
    attn = softmax(sim); out = (attn @ v) @ Wout + bout

Sharding (8 cores): core c -> batch b = c//2, parity r = c%2.
Each core owns the 16 query blocks g = 2t + r (t = 0..15) of its batch,
giving every core an identical static program: slot t processes key
columns [0, 256*(t+1)) -- the exact causal width for odd g, one padded
128-block for even g. Total key-block work per core is identical (272).

The causal mask is baked into a host-packed `prev` (masked entries hold
-FLT_MAX; adding q.k to -FLT_MAX is absorbed exactly in fp32, and
exp(-FLT_MAX) = 0), so the device applies no mask at all.

Weight folds done on host: SCALE into Wq, and Wv @ Wout into one matrix
(attn @ (v @ Wout) == (attn @ v) @ Wout mathematically).

Per-core device program (all matmuls fp32r = full PE rate, ~1.2e-4 rel):
  P0: qT = (xq @ Wq*scale)^T resident in SBUF; kT = (x @ Wk)^T;
      vW = x @ (Wv@Wout).
  P1 per slot: sim chunks = qT.T @ kT + prev (DVE add), DMA'd out;
      exp on ACT with accum_out row-sums; PE transpose of attn chunks;
      attn^T @ vW accumulated in PSUM; final = avp * (1/rowsum) + bout.
      Fully-masked tail of each sim row written from a -FLT_MAX tile.
Stores issue from the gpsimd queue, loads from sync, to split the DMA
descriptor-issue cost across sequencers.
"""

import os
import sys

sys.path.insert(0, "/opt/trn_rl_repo")

from contextlib import ExitStack

import numpy as np

import concourse.bacc as bacc
import concourse.bass as bass
import concourse.mybir as mybir
import concourse.tile as tile
from concourse.bass_utils import run_bass_kernel_spmd
from concourse.masks import make_identity

F32 = mybir.dt.float32
F32R = mybir.dt.float32r
AF = mybir.ActivationFunctionType
ALU = mybir.AluOpType

B, N, D = 4, 4096, 512
NB = 128          # query/key block
T = 16            # slots (query blocks) per core
SCALE = D ** -0.5
NEG_MAX = float(-np.finfo(np.float32).max)

SLOT_W = [256 * (t + 1) for t in range(T)]            # key width per slot
SLOT_OFF = np.concatenate([[0], np.cumsum([NB * w for w in SLOT_W])])
PACKED = int(SLOT_OFF[-1])                            # prevp element count

LAST_EXEC_NS = None


def _chunks(w):
    """Split [0, w) into 512-wide chunks (last may be 256)."""
    return [(off, min(512, w - off)) for off in range(0, w, 512)]


def build_nc():
    nc = bacc.Bacc("TRN2", target_bir_lowering=False)

    xT = nc.declare_dram_parameter("xT", [D, N], F32R, isOutput=False)
    xq = nc.declare_dram_parameter("xq", [D, T * NB], F32R, isOutput=False)
    wq = nc.declare_dram_parameter("wq", [D, D], F32R, isOutput=False)
    wk = nc.declare_dram_parameter("wk", [D, D], F32R, isOutput=False)
    wf = nc.declare_dram_parameter("wf", [D, D], F32R, isOutput=False)
    prevp = nc.declare_dram_parameter("prevp", [PACKED], F32, isOutput=False)
    boutb = nc.declare_dram_parameter("boutb", [NB, D], F32, isOutput=False)
    simo = nc.declare_dram_parameter("simo", [T, NB, N], F32, isOutput=True)
    outo = nc.declare_dram_parameter("outo", [T, NB, D], F32, isOutput=True)

    with ExitStack() as ctx:
        tc = ctx.enter_context(tile.TileContext(nc))

        # ---- persistent pools ----
        kp = ctx.enter_context(tc.tile_pool(name="kp", bufs=1))
        kt = [kp.tile([NB, N], F32R, tag=f"kt{i}", name=f"kt{i}") for i in range(4)]
        vp = ctx.enter_context(tc.tile_pool(name="vp", bufs=1))
        vw = [vp.tile([NB, D], F32R, tag=f"vw{j}", name=f"vw{j}") for j in range(32)]
        qp = ctx.enter_context(tc.tile_pool(name="qp", bufs=1))
        qt = [qp.tile([NB, T * NB], F32R, tag=f"qt{i}", name=f"qt{i}") for i in range(4)]
        cp = ctx.enter_context(tc.tile_pool(name="cp", bufs=1))
        ident = cp.tile([NB, NB], F32)
        identr = cp.tile([NB, NB], F32R)

        # ---- phase 0: projections (merged; weights coexist, x loads on gpsimd) ----
        with (
            tc.tile_pool(name="wp", bufs=1) as wp,
            tc.tile_pool(name="xp", bufs=8) as xp,
            tc.tile_pool(name="p0", bufs=8, space="PSUM") as p0,
        ):
            wq_sb = [wp.tile([NB, D], F32R, tag=f"wq{i}", name=f"wq{i}") for i in range(4)]
            for i in range(4):
                nc.sync.dma_start(out=wq_sb[i], in_=wq[i * NB : (i + 1) * NB, :])
            wk_sb = [wp.tile([NB, D], F32R, tag=f"wk{i}", name=f"wk{i}") for i in range(4)]
            wf_sb = [wp.tile([NB, D], F32R, tag=f"wf{i}", name=f"wf{i}") for i in range(4)]
            for i in range(4):
                nc.sync.dma_start(out=wk_sb[i], in_=wk[i * NB : (i + 1) * NB, :])
                nc.sync.dma_start(out=wf_sb[i], in_=wf[i * NB : (i + 1) * NB, :])

            # qT (stays resident in SBUF)
            for qc in range(4):
                xqc = []
                for dk in range(4):
                    xc = xp.tile([NB, 512], F32R, tag="xc", name=f"xqc{qc}_{dk}")
                    nc.gpsimd.dma_start(
                        out=xc,
                        in_=xq[dk * NB : (dk + 1) * NB, qc * 512 : (qc + 1) * 512],
                    )
                    xqc.append(xc)
                for dt_ in range(4):
                    ps = p0.tile([NB, 512], F32, tag="p0", name=f"pq{qc}_{dt_}")
                    for dk in range(4):
                        nc.tensor.matmul(
                            ps,
                            wq_sb[dk][:, dt_ * NB : (dt_ + 1) * NB],
                            xqc[dk],
                            start=(dk == 0),
                            stop=(dk == 3),
                        )
                    nc.scalar.activation(
                        out=qt[dt_][:, qc * 512 : (qc + 1) * 512], in_=ps, func=AF.Copy
                    )

            # kT and vW from streamed xT chunks
            for ic in range(8):
                xtc = []
                for dk in range(4):
                    xc = xp.tile([NB, 512], F32R, tag="xc", name=f"xtc{ic}_{dk}")
                    nc.gpsimd.dma_start(
                        out=xc,
                        in_=xT[dk * NB : (dk + 1) * NB, ic * 512 : (ic + 1) * 512],
                    )
                    xtc.append(xc)
                for dt_ in range(4):
                    ps = p0.tile([NB, 512], F32, tag="p0", name=f"pk{ic}_{dt_}")
                    for dk in range(4):
                        nc.tensor.matmul(
                            ps,
                            wk_sb[dk][:, dt_ * NB : (dt_ + 1) * NB],
                            xtc[dk],
                            start=(dk == 0),
                            stop=(dk == 3),
                        )
                    nc.scalar.activation(
                        out=kt[dt_][:, ic * 512 : (ic + 1) * 512], in_=ps, func=AF.Copy
                    )
                for jb in range(4):
                    ps = p0.tile([NB, 512], F32, tag="p0", name=f"pv{ic}_{jb}")
                    for dk in range(4):
                        nc.tensor.matmul(
                            ps,
                            xtc[dk][:, jb * NB : (jb + 1) * NB],
                            wf_sb[dk],
                            start=(dk == 0),
                            stop=(dk == 3),
                        )
                    nc.vector.tensor_copy(out=vw[ic * 4 + jb], in_=ps)

        # phase-1-only constants (late so their setup doesn't clog startup)
        make_identity(nc, ident)
        nc.scalar.activation(out=identr, in_=ident, func=AF.Copy)
        bout_sb = cp.tile([NB, D], F32)
        nc.sync.dma_start(out=bout_sb, in_=boutb[:, :])
        fmax_sb = cp.tile([NB, 1024], F32)
        nc.gpsimd.memset(fmax_sb, NEG_MAX)

        # ---- phase 1: attention slots ----
        with (
            tc.tile_pool(name="pvp", bufs=4) as pvp,
            tc.tile_pool(name="scp", bufs=4) as scp,
            tc.tile_pool(name="atp", bufs=3) as atp,
            tc.tile_pool(name="ttp", bufs=3) as ttp,
            tc.tile_pool(name="fip", bufs=2) as fip,
            tc.tile_pool(name="smp", bufs=2) as smp,
            tc.tile_pool(name="ps1", bufs=3, space="PSUM") as ps1,
            tc.tile_pool(name="ps2", bufs=2, space="PSUM") as ps2,
            tc.tile_pool(name="ps3", bufs=2, space="PSUM") as ps3,
        ):
            for t in reversed(range(T)):
                w = SLOT_W[t]
                nblk = w // NB
                qsl = [qt[dk][:, t * NB : (t + 1) * NB] for dk in range(4)]

                avp = ps3.tile([NB, D], F32, tag="avp", name=f"avp{t}")
                acc = smp.tile([NB, 8], F32, tag="acc", name=f"acc{t}")
                ch = _chunks(w)
                pview = prevp[
                    int(SLOT_OFF[t]) : int(SLOT_OFF[t]) + NB * w
                ].rearrange("(p w) -> p w", p=NB)

                for ci, (off, cw) in enumerate(ch):
                    simp = ps1.tile([NB, 512], F32, tag="simp", name=f"sp{t}_{ci}")
                    for dk in range(4):
                        nc.tensor.matmul(
                            simp[:, :cw],
                            qsl[dk],
                            kt[dk][:, off : off + cw],
                            start=(dk == 0),
                            stop=(dk == 3),
                        )
                    pv = pvp.tile([NB, 512], F32, tag="pv", name=f"pv{t}_{ci}")
                    nc.sync.dma_start(out=pv[:, :cw], in_=pview[:, off : off + cw])
                    sc = scp.tile([NB, 512], F32, tag="sc", name=f"sc{t}_{ci}")
                    nc.vector.tensor_tensor(
                        out=sc[:, :cw], in0=simp[:, :cw], in1=pv[:, :cw], op=ALU.add
                    )
                    nc.gpsimd.dma_start(
                        out=simo[t, :, off : off + cw], in_=sc[:, :cw]
                    )
                    at = atp.tile([NB, 512], F32R, tag="at", name=f"at{t}_{ci}")
                    nc.scalar.activation(
                        out=at[:, :cw],
                        in_=sc[:, :cw],
                        func=AF.Exp,
                        accum_out=acc[:, ci : ci + 1],
                    )
                    trp = ps2.tile([NB, 512], F32R, tag="trp", name=f"tp{t}_{ci}")
                    for bi in range(cw // NB):
                        nc.tensor.transpose(
                            trp[:, bi * NB : (bi + 1) * NB],
                            at[:, bi * NB : (bi + 1) * NB],
                            identr,
                        )
                    att = ttp.tile([NB, 512], F32R, tag="att", name=f"att{t}_{ci}")
                    nc.vector.tensor_copy(out=att[:, :cw], in_=trp[:, :cw])
                    for bi in range(cw // NB):
                        jb = off // NB + bi
                        nc.tensor.matmul(
                            avp,
                            att[:, bi * NB : (bi + 1) * NB],
                            vw[jb],
                            start=(jb == 0),
                            stop=(jb == nblk - 1),
                        )

                rtot = smp.tile([NB, 1], F32, tag="rtot", name=f"rt{t}")
                nc.vector.reduce_sum(
                    out=rtot, in_=acc[:, : len(ch)], axis=mybir.AxisListType.X
                )
                rec = smp.tile([NB, 1], F32, tag="rec", name=f"rc{t}")
                nc.vector.reciprocal(out=rec, in_=rtot)
                fin = fip.tile([NB, D], F32, tag="fin", name=f"fin{t}")
                nc.vector.scalar_tensor_tensor(
                    out=fin,
                    in0=avp,
                    scalar=rec,
                    in1=bout_sb,
                    op0=ALU.mult,
                    op1=ALU.add,
                )
                nc.gpsimd.dma_start(out=outo[t, :, :], in_=fin)

                # fully-masked tail of the sim rows
                pos = w
                while pos < N:
                    seg = min(N - pos, 1024)
                    nc.gpsimd.dma_start(
                        out=simo[t, :, pos : pos + seg], in_=fmax_sb[:, :seg]
                    )
                    pos += seg

    nc.compile()
    return nc


_NC_CACHE = None


def _get_nc():
    global _NC_CACHE
    if _NC_CACHE is None:
        _NC_CACHE = build_nc()
    return _NC_CACHE


def _pack_prev(prev_b, r):
    """Pack one core's causal prev slices (mask baked in) into a flat array."""
    out = np.empty(PACKED, dtype=np.float32)
    triu = np.triu(np.ones((NB, NB), dtype=bool), k=1)
    for t in range(T):
        g = 2 * t + r
        w = SLOT_W[t]
        causal = (g + 1) * NB
        blk = np.empty((NB, w), dtype=np.float32)
        m = min(causal, w)
        blk[:, :m] = prev_b[g * NB : (g + 1) * NB, :m]
        if w > causal:
            blk[:, causal:] = NEG_MAX
        ds = g * NB  # diagonal block column start (always < w)
        dblk = blk[:, ds : ds + NB]
        dblk[triu] = NEG_MAX
        out[SLOT_OFF[t] : SLOT_OFF[t + 1]] = blk.ravel()
    return out


def kernel(x, prev, Wqkv, Wout, bout):
    global LAST_EXEC_NS
    x = np.asarray(x, dtype=np.float32)
    prev = np.asarray(prev, dtype=np.float32)
    Wqkv = np.asarray(Wqkv, dtype=np.float32)
    Wout = np.asarray(Wout, dtype=np.float32)
    bout = np.asarray(bout, dtype=np.float32)

    wq = np.ascontiguousarray(Wqkv[:, :D] * np.float32(SCALE))
    wk = np.ascontiguousarray(Wqkv[:, D : 2 * D])
    wv = Wqkv[:, 2 * D :]
    wf = (wv.astype(np.float64) @ Wout.astype(np.float64)).astype(np.float32)
    boutb = np.ascontiguousarray(np.broadcast_to(bout, (NB, D)))

    in_maps = []
    for c in range(8):
        b, r = c // 2, c % 2
        rows = np.arange(T) * 2 + r  # owned query blocks
        qidx = (rows[:, None] * NB + np.arange(NB)[None, :]).ravel()
        in_maps.append(
            {
                "xT": np.ascontiguousarray(x[b].T),
                "xq": np.ascontiguousarray(x[b][qidx].T),
                "wq": wq,
                "wk": wk,
                "wf": wf,
                "prevp": _pack_prev(prev[b], r),
                "boutb": boutb,
            }
        )

    nc = _get_nc()
    trace = bool(os.environ.get("BASSKERNEL_TRACE"))
    res = run_bass_kernel_spmd(nc, in_maps, list(range(8)), trace=trace)
    LAST_EXEC_NS = res.exec_time_ns

    sim = np.empty((B, N, N), dtype=np.float32)
    out = np.empty((B, N, D), dtype=np.float32)
    for c in range(8):
        b, r = c // 2, c % 2
        so = res.results[c]["simo"]
        oo = res.results[c]["outo"]
        for t in range(T):
            g = 2 * t + r
            sim[b, g * NB : (g + 1) * NB, :] = so[t]
            out[b, g * NB : (g + 1) * NB, :] = oo[t]
    return (out, sim)


# revision 8
# speedup vs baseline: 1.0225x; 1.0225x over previous
"""Trainium2 Bass kernel for nn_Attention_65403761984268.

Causal attention with additive score bias, returning (out, sim):
    qkv = x @ Wqkv ; q,k,v = split(qkv)
    sim = q @ k^T * d^-0.5 + prev, masked (strict upper) with -FLT_MAX
    attn = softmax(sim); out = (attn @ v) @ Wout + bout

Sharding (8 cores): core c -> batch b = c//2, parity r = c%2.
Each core owns the 16 query blocks g = 2t + r (t = 0..15) of its batch,
giving every core an identical static program: slot t processes key
columns [0, 256*(t+1)) -- the exact causal width for odd g, one padded
128-block for even g. Total key-block work per core is identical (272).

The causal mask is baked into a host-packed `prev` (masked entries hold
-FLT_MAX; adding q.k to -FLT_MAX is absorbed exactly in fp32, and
exp(-FLT_MAX) = 0), so the device applies no mask at all.

Weight folds done on host: SCALE into Wq, and Wv @ Wout into one matrix
(attn @ (v @ Wout) == (attn @ v) @ Wout mathematically).

Per-core device program (all matmuls fp32r = full PE rate, ~1.2e-4 rel):
  P0: qT = (xq @ Wq*scale)^T resident in SBUF; kT = (x @ Wk)^T;
      vW = x @ (Wv@Wout).
  P1 per slot: sim chunks = qT.T @ kT + prev (DVE add), DMA'd out;
      exp on ACT with accum_out row-sums; PE transpose of attn chunks;
      attn^T @ vW accumulated in PSUM; final = avp * (1/rowsum) + bout.
      Fully-masked tail of each sim row written from a -FLT_MAX tile.
Stores issue from the gpsimd queue, loads from sync, to split the DMA
descriptor-issue cost across sequencers.
"""

import os
import sys

sys.path.insert(0, "/opt/trn_rl_repo")

from contextlib import ExitStack

import numpy as np

import concourse.bacc as bacc
import concourse.bass as bass
import concourse.mybir as mybir
import concourse.tile as tile
from concourse.bass_utils import run_bass_kernel_spmd
from concourse.masks import make_identity

F32 = mybir.dt.float32
F32R = mybir.dt.float32r
AF = mybir.ActivationFunctionType
ALU = mybir.AluOpType

B, N, D = 4, 4096, 512
NB = 128          # query/key block
T = 16            # slots (query blocks) per core
SCALE = D ** -0.5
NEG_MAX = float(-np.finfo(np.float32).max)

SLOT_W = [256 * (t + 1) for t in range(T)]            # key width per slot
SLOT_OFF = np.concatenate([[0], np.cumsum([NB * w for w in SLOT_W])])
PACKED = int(SLOT_OFF[-1])                            # prevp element count

LAST_EXEC_NS = None


def _chunks(w):
    """Split [0, w) into 512-wide chunks (last may be 256)."""
    return [(off, min(512, w - off)) for off in range(0, w, 512)]


def build_nc():
    nc = bacc.Bacc("TRN2", target_bir_lowering=False)

    xT = nc.declare_dram_parameter("xT", [D, N], F32R, isOutput=False)
    xq = nc.declare_dram_parameter("xq", [D, T * NB], F32R, isOutput=False)
    wq = nc.declare_dram_parameter("wq", [D, D], F32R, isOutput=False)
    wk = nc.declare_dram_parameter("wk", [D, D], F32R, isOutput=False)
    wf = nc.declare_dram_parameter("wf", [D, D], F32R, isOutput=False)
    prevp = nc.declare_dram_parameter("prevp", [PACKED], F32, isOutput=False)
    boutb = nc.declare_dram_parameter("boutb", [NB, D], F32, isOutput=False)
    simo = nc.declare_dram_parameter("simo", [T, NB, N], F32, isOutput=True)
    outo = nc.declare_dram_parameter("outo", [T, NB, D], F32, isOutput=True)

    with ExitStack() as ctx:
        tc = ctx.enter_context(tile.TileContext(nc))

        # ---- persistent pools ----
        kp = ctx.enter_context(tc.tile_pool(name="kp", bufs=1))
        kt = [kp.tile([NB, N], F32R, tag=f"kt{i}", name=f"kt{i}") for i in range(4)]
        vp = ctx.enter_context(tc.tile_pool(name="vp", bufs=1))
        vw = [vp.tile([NB, D], F32R, tag=f"vw{j}", name=f"vw{j}") for j in range(32)]
        qp = ctx.enter_context(tc.tile_pool(name="qp", bufs=1))
        qt = [qp.tile([NB, T * NB], F32R, tag=f"qt{i}", name=f"qt{i}") for i in range(4)]
        cp = ctx.enter_context(tc.tile_pool(name="cp", bufs=1))
        ident = cp.tile([NB, NB], F32)
        identr = cp.tile([NB, NB], F32R)

        # ---- phase 0: projections (merged; weights coexist, x loads on gpsimd) ----
        with (
            tc.tile_pool(name="wp", bufs=1) as wp,
            tc.tile_pool(name="xp", bufs=8) as xp,
            tc.tile_pool(name="p0", bufs=8, space="PSUM") as p0,
        ):
            wq_sb = [wp.tile([NB, D], F32R, tag=f"wq{i}", name=f"wq{i}") for i in range(4)]
            for i in range(4):
                nc.sync.dma_start(out=wq_sb[i], in_=wq[i * NB : (i + 1) * NB, :])
            wk_sb = [wp.tile([NB, D], F32R, tag=f"wk{i}", name=f"wk{i}") for i in range(4)]
            wf_sb = [wp.tile([NB, D], F32R, tag=f"wf{i}", name=f"wf{i}") for i in range(4)]
            for i in range(4):
                nc.sync.dma_start(out=wk_sb[i], in_=wk[i * NB : (i + 1) * NB, :])
                nc.sync.dma_start(out=wf_sb[i], in_=wf[i * NB : (i + 1) * NB, :])

            # qT (stays resident in SBUF)
            for qc in range(4):
                xqc = []
                for dk in range(4):
                    xc = xp.tile([NB, 512], F32R, tag="xc", name=f"xqc{qc}_{dk}")
                    nc.gpsimd.dma_start(
                        out=xc,
                        in_=xq[dk * NB : (dk + 1) * NB, qc * 512 : (qc + 1) * 512],
                    )
                    xqc.append(xc)
                for dt_ in range(4):
                    ps = p0.tile([NB, 512], F32, tag="p0", name=f"pq{qc}_{dt_}")
                    for dk in range(4):
                        nc.tensor.matmul(
                            ps,
                            wq_sb[dk][:, dt_ * NB : (dt_ + 1) * NB],
                            xqc[dk],
                            start=(dk == 0),
                            stop=(dk == 3),
                        )
                    nc.scalar.activation(
                        out=qt[dt_][:, qc * 512 : (qc + 1) * 512], in_=ps, func=AF.Copy
                    )

            # kT and vW from streamed xT chunks
            for ic in range(8):
                xtc = []
                for dk in range(4):
                    xc = xp.tile([NB, 512], F32R, tag="xc", name=f"xtc{ic}_{dk}")
                    nc.gpsimd.dma_start(
                        out=xc,
                        in_=xT[dk * NB : (dk + 1) * NB, ic * 512 : (ic + 1) * 512],
                    )
                    xtc.append(xc)
                for dt_ in range(4):
                    ps = p0.tile([NB, 512], F32, tag="p0", name=f"pk{ic}_{dt_}")
                    for dk in range(4):
                        nc.tensor.matmul(
                            ps,
                            wk_sb[dk][:, dt_ * NB : (dt_ + 1) * NB],
                            xtc[dk],
                            start=(dk == 0),
                            stop=(dk == 3),
                        )
                    nc.scalar.activation(
                        out=kt[dt_][:, ic * 512 : (ic + 1) * 512], in_=ps, func=AF.Copy
                    )
                for jb in range(4):
                    ps = p0.tile([NB, 512], F32, tag="p0", name=f"pv{ic}_{jb}")
                    for dk in range(4):
                        nc.tensor.matmul(
                            ps,
                            xtc[dk][:, jb * NB : (jb + 1) * NB],
                            wf_sb[dk],
                            start=(dk == 0),
                            stop=(dk == 3),
                        )
                    nc.vector.tensor_copy(out=vw[ic * 4 + jb], in_=ps)

        # phase-1-only constants (late so their setup doesn't clog startup)
        make_identity(nc, ident)
        nc.scalar.activation(out=identr, in_=ident, func=AF.Copy)
        bout_sb = cp.tile([NB, D], F32)
        nc.sync.dma_start(out=bout_sb, in_=boutb[:, :])
        fmax_sb = cp.tile([NB, 1024], F32)
        nc.gpsimd.memset(fmax_sb, NEG_MAX)
        # fully-masked tails of all sim rows, streamed early (independent of
        # all compute; fills otherwise-idle store bandwidth during phase 1)
        for t in range(T):
            pos = SLOT_W[t]
            while pos < N:
                seg = min(N - pos, 1024)
                nc.gpsimd.dma_start(
                    out=simo[t, :, pos : pos + seg], in_=fmax_sb[:, :seg]
                )
                pos += seg

        # ---- phase 1: attention slots ----
        with (
            tc.tile_pool(name="pvp", bufs=4) as pvp,
            tc.tile_pool(name="scp", bufs=4) as scp,
            tc.tile_pool(name="atp", bufs=3) as atp,
            tc.tile_pool(name="ttp", bufs=3) as ttp,
            tc.tile_pool(name="fip", bufs=2) as fip,
            tc.tile_pool(name="smp", bufs=2) as smp,
            tc.tile_pool(name="ps1", bufs=3, space="PSUM") as ps1,
            tc.tile_pool(name="ps2", bufs=3, space="PSUM") as ps2,
            tc.tile_pool(name="ps3", bufs=2, space="PSUM") as ps3,
        ):
            for t in reversed(range(T)):
                w = SLOT_W[t]
                nblk = w // NB
                qsl = [qt[dk][:, t * NB : (t + 1) * NB] for dk in range(4)]

                avp = ps3.tile([NB, D], F32, tag="avp", name=f"avp{t}")
                acc = smp.tile([NB, 8], F32, tag="acc", name=f"acc{t}")
                ch = _chunks(w)
                pview = prevp[
                    int(SLOT_OFF[t]) : int(SLOT_OFF[t]) + NB * w
                ].rearrange("(p w) -> p w", p=NB)

                for ci, (off, cw) in enumerate(ch):
                    simp = ps1.tile([NB, 512], F32, tag="simp", name=f"sp{t}_{ci}")
                    for dk in range(4):
                        nc.tensor.matmul(
                            simp[:, :cw],
                            qsl[dk],
                            kt[dk][:, off : off + cw],
                            start=(dk == 0),
                            stop=(dk == 3),
                        )
                    pv = pvp.tile([NB, 512], F32, tag="pv", name=f"pv{t}_{ci}")
                    nc.sync.dma_start(out=pv[:, :cw], in_=pview[:, off : off + cw])
                    sc = scp.tile([NB, 512], F32, tag="sc", name=f"sc{t}_{ci}")
                    nc.vector.tensor_tensor(
                        out=sc[:, :cw], in0=simp[:, :cw], in1=pv[:, :cw], op=ALU.add
                    )
                    nc.gpsimd.dma_start(
                        out=simo[t, :, off : off + cw], in_=sc[:, :cw]
                    )
                    at = atp.tile([NB, 512], F32R, tag="at", name=f"at{t}_{ci}")
                    nc.scalar.activation(
                        out=at[:, :cw],
                        in_=sc[:, :cw],
                        func=AF.Exp,
                        accum_out=acc[:, ci : ci + 1],
                    )
                    trp = ps2.tile([NB, 512], F32R, tag="trp", name=f"tp{t}_{ci}")
                    for bi in range(cw // NB):
                        nc.tensor.transpose(
                            trp[:, bi * NB : (bi + 1) * NB],
                            at[:, bi * NB : (bi + 1) * NB],
                            identr,
                        )
                    att = ttp.tile([NB, 512], F32R, tag="att", name=f"att{t}_{ci}")
                    nc.vector.tensor_copy(out=att[:, :cw], in_=trp[:, :cw])
                    for bi in range(cw // NB):
                        jb = off // NB + bi
                        nc.tensor.matmul(
                            avp,
                            att[:, bi * NB : (bi + 1) * NB],
                            vw[jb],
                            start=(jb == 0),
                            stop=(jb == nblk - 1),
                        )

                rtot = smp.tile([NB, 1], F32, tag="rtot", name=f"rt{t}")
                nc.vector.reduce_sum(
                    out=rtot, in_=acc[:, : len(ch)], axis=mybir.AxisListType.X
                )
                rec = smp.tile([NB, 1], F32, tag="rec", name=f"rc{t}")
                nc.vector.reciprocal(out=rec, in_=rtot)
                fin = fip.tile([NB, D], F32, tag="fin", name=f"fin{t}")
                nc.vector.scalar_tensor_tensor(
                    out=fin,
                    in0=avp,
                    scalar=rec,
                    in1=bout_sb,
                    op0=ALU.mult,
                    op1=ALU.add,
                )
                nc.gpsimd.dma_start(out=outo[t, :, :], in_=fin)


    nc.compile()
    return nc


_NC_CACHE = None


def _get_nc():
    global _NC_CACHE
    if _NC_CACHE is None:
        _NC_CACHE = build_nc()
    return _NC_CACHE


def _pack_prev(prev_b, r):
    """Pack one core's causal prev slices (mask baked in) into a flat array."""
    out = np.empty(PACKED, dtype=np.float32)
    triu = np.triu(np.ones((NB, NB), dtype=bool), k=1)
    for t in range(T):
        g = 2 * t + r
        w = SLOT_W[t]
        causal = (g + 1) * NB
        blk = np.empty((NB, w), dtype=np.float32)
        m = min(causal, w)
        blk[:, :m] = prev_b[g * NB : (g + 1) * NB, :m]
        if w > causal:
            blk[:, causal:] = NEG_MAX
        ds = g * NB  # diagonal block column start (always < w)
        dblk = blk[:, ds : ds + NB]
        dblk[triu] = NEG_MAX
        out[SLOT_OFF[t] : SLOT_OFF[t + 1]] = blk.ravel()
    return out


def kernel(x, prev, Wqkv, Wout, bout):
    global LAST_EXEC_NS
    x = np.asarray(x, dtype=np.float32)
    prev = np.asarray(prev, dtype=np.float32)
    Wqkv = np.asarray(Wqkv, dtype=np.float32)
    Wout = np.asarray(Wout, dtype=np.float32)
    bout = np.asarray(bout, dtype=np.float32)

    wq = np.ascontiguousarray(Wqkv[:, :D] * np.float32(SCALE))
    wk = np.ascontiguousarray(Wqkv[:, D : 2 * D])
    wv = Wqkv[:, 2 * D :]
    wf = (wv.astype(np.float64) @ Wout.astype(np.float64)).astype(np.float32)
    boutb = np.ascontiguousarray(np.broadcast_to(bout, (NB, D)))

    in_maps = []
    for c in range(8):
        b, r = c // 2, c % 2
        rows = np.arange(T) * 2 + r  # owned query blocks
        qidx = (rows[:, None] * NB + np.arange(NB)[None, :]).ravel()
        in_maps.append(
            {
                "xT": np.ascontiguousarray(x[b].T),
                "xq": np.ascontiguousarray(x[b][qidx].T),
                "wq": wq,
                "wk": wk,
                "wf": wf,
                "prevp": _pack_prev(prev[b], r),
                "boutb": boutb,
            }
        )

    nc = _get_nc()
    trace = bool(os.environ.get("BASSKERNEL_TRACE"))
    res = run_bass_kernel_spmd(nc, in_maps, list(range(8)), trace=trace)
    LAST_EXEC_NS = res.exec_time_ns

    sim = np.empty((B, N, N), dtype=np.float32)
    out = np.empty((B, N, D), dtype=np.float32)
    for c in range(8):
        b, r = c // 2, c % 2
        so = res.results[c]["simo"]
        oo = res.results[c]["outo"]
        for t in range(T):
            g = 2 * t + r
            sim[b, g * NB : (g + 1) * NB, :] = so[t]
            out[b, g * NB : (g + 1) * NB, :] = oo[t]
    return (out, sim)
